# revision 13
# baseline (speedup 1.0000x reference)
"""Trainium2 Bass kernel for single-token causal self-attention with KV cache.

Problem (hardcoded per spec): B=32, S=2048 cached positions, D=128 head dim
(= d_model), H=16 heads.  Returns (out, k, v) like the reference.

Sharding: data parallel over batch, 4 batches/core on 8 cores; no cross-core
communication.

Device kernel structure (per core, 64 (head,batch) pairs):
  - One contiguous 2.06 MB DMA per pair brings in K chunked (s on partitions)
    and V chunked with a ones-column appended per chunk (for the softmax
    denominator).
  - Scores on the VectorEngine: elementwise K * broadcast(q) (fp32, exact)
    then a per-chunk free-dim reduce -> scores (128 x 16).  This keeps the
    K pass off the TensorEngine, whose fp32 weight-load path is the
    bottleneck (~150 GB/s).
  - exp on ScalarE; V pass on TensorE with probs as the (1-column) stationary
    operand and V streaming through the moving side (~300 GB/s fp32), giving
    unnormalized y rows plus the denominator via the ones column.
  - A 1-column outer-product matmul transposes each y row into a column
    accumulator; a vectorized epilogue folds in the new token exactly
    (e_new, v_new from host), normalizes, out-projects, adds the residual
    and applies layernorm.
The tiny projections (qkv of one token, 25 MFLOP) and the k/v cache
concatenation are host-side numpy; all S-scaling work runs on device.
"""

import math

import numpy as np

import concourse.bacc as bacc
import concourse.bass as bass
import concourse.tile as tile
from concourse import mybir
from concourse.bass_utils import run_bass_kernel_spmd

F32 = mybir.dt.float32
AF = mybir.ActivationFunctionType
ALU = mybir.AluOpType
AX = mybir.AxisListType

# Problem dims
B, S, D, H = 32, 2048, 128, 16
N_CORES = 8
BL = B // N_CORES          # local batches per core
PAIRS = H * BL             # (head, batch) pairs per core
NS = S // 128              # 128-wide chunks of cached sequence
EPS = 1e-5


def build_program(bl=BL, h=H, ns=NS):
    """Emit the per-core Tile program. Parameterized so a scaled-down variant
    can be checked in CoreSim."""
    pairs = h * bl
    s = ns * 128
    sv = s + ns * 129          # K region | V_ext region (129-wide chunks)

    nc = bacc.Bacc(
        "TRN2", target_bir_lowering=False, debug=False, num_devices=N_CORES
    )

    kvblob = nc.dram_tensor("kvblob", [pairs, 128, sv], F32, kind="ExternalInput").ap()
    q16 = nc.dram_tensor("q16", [pairs, 128], F32, kind="ExternalInput").ap()
    vnewT = nc.dram_tensor("vnewT", [128, pairs], F32, kind="ExternalInput").ap()
    enew = nc.dram_tensor("enew", [1, pairs], F32, kind="ExternalInput").ap()
    woutT = nc.dram_tensor("woutT", [128, h * 128], F32, kind="ExternalInput").ap()
    xpb = nc.dram_tensor("xpb", [bl, 128], F32, kind="ExternalInput").ap()
    out_d = nc.dram_tensor("out", [bl, 128], F32, kind="ExternalOutput").ap()

    with tile.TileContext(nc) as tc:
        with (
            tc.tile_pool(name="const", bufs=1) as const,
            tc.tile_pool(name="kv", bufs=7) as kvpool,
            tc.tile_pool(name="qb", bufs=3) as qbpool,
            tc.tile_pool(name="pd", bufs=2) as pdpool,
            tc.tile_pool(name="pr", bufs=3) as prpool,
            tc.tile_pool(name="epi", bufs=1) as epi,
            tc.tile_pool(name="ypsum", bufs=3, space="PSUM") as ypsum,
            tc.tile_pool(name="mpsum", bufs=2, space="PSUM") as mpsum,
            tc.tile_pool(name="apsum", bufs=1, space="PSUM") as apsum,
        ):
            # ---- constants / small inputs (SWDGE: keep the HWDGE ring for
            # the big blob loads) ----
            q16_sb = const.tile([1, pairs * 128], F32)
            nc.gpsimd.dma_start(out=q16_sb, in_=q16.rearrange("p d -> (p d)")[None, :])
            woutT_sb = const.tile([128, h * 128], F32)
            nc.gpsimd.dma_start(out=woutT_sb, in_=woutT)
            xpb_sb = const.tile([bl, 128], F32)
            nc.gpsimd.dma_start(out=xpb_sb, in_=xpb)
            vnewT_sb = const.tile([128, pairs], F32)
            nc.gpsimd.dma_start(out=vnewT_sb, in_=vnewT)
            enew_sb = const.tile([1, pairs], F32)
            nc.gpsimd.dma_start(out=enew_sb, in_=enew)
            ones_one = const.tile([1, 1], F32)
            nc.vector.memset(ones_one, 1.0)
            ones_row = const.tile([1, 128], F32)
            nc.vector.memset(ones_row, 1.0)
            eps_sb = const.tile([bl, 1], F32)
            nc.vector.memset(eps_sb, EPS)

            # unnormalized y rows + denominators, one 129-slot per pair
            yall = const.tile([1, pairs, 129], F32)
            # y^T columns accumulator
            attnT_ps = apsum.tile([128, pairs], F32, tag="attnT")

            attn_sb = const.tile([128, pairs], F32)
            oproj_ps = apsum.tile([bl, 128], F32, tag="oproj")

            def normalize_heads(h0, h1):
                """Fold in new token + normalize + out-project heads [h0,h1)."""
                lo, hi = h0 * bl, h1 * bl
                n = hi - lo
                dtot = epi.tile([1, n], F32, tag="dtot")
                nc.vector.tensor_add(dtot, yall[0:1, lo:hi, 128], enew_sb[0:1, lo:hi])
                brsrc = epi.tile([1, 2 * n], F32, tag="brsrc")
                nc.vector.reciprocal(brsrc[0:1, 0:n], dtot)
                nc.vector.tensor_mul(
                    brsrc[0:1, n : 2 * n], enew_sb[0:1, lo:hi], brsrc[0:1, 0:n]
                )
                bc_ps = mpsum.tile([128, 2 * n], F32, tag="m")
                nc.tensor.matmul(bc_ps, lhsT=ones_row, rhs=brsrc, start=True, stop=True)
                bc_sb = epi.tile([128, 2 * n], F32, tag="bc")
                nc.vector.tensor_copy(bc_sb, bc_ps)
                t1 = epi.tile([128, n], F32, tag="t1")
                nc.vector.tensor_mul(t1, attnT_ps[:, lo:hi], bc_sb[:, 0:n])
                t2 = epi.tile([128, n], F32, tag="t2")
                nc.vector.tensor_mul(t2, vnewT_sb[:, lo:hi], bc_sb[:, n : 2 * n])
                nc.vector.tensor_add(attn_sb[:, lo:hi], t1, t2)
                for hh in range(h0, h1):
                    nc.tensor.matmul(
                        oproj_ps,
                        lhsT=attn_sb[:, hh * bl : (hh + 1) * bl],
                        rhs=woutT_sb[:, hh * 128 : (hh + 1) * 128],
                        start=(hh == 0),
                        stop=(hh == h - 1),
                    )

            # ---- main loop over (head, batch) pairs ----
            for p in range(pairs):
                kv = kvpool.tile([128, sv], F32, tag="kv")
                if p >= pairs - 2:
                    # latency-split the last loads so compute drains sooner
                    nc.sync.dma_start(out=kv[:, 0:s], in_=kvblob[p, :, 0:s])
                    nc.sync.dma_start(out=kv[:, s:sv], in_=kvblob[p, :, s:sv])
                else:
                    nc.sync.dma_start(out=kv, in_=kvblob[p])

                # q broadcast from partition 0 to all partitions (GpSimd)
                qbc = qbpool.tile([128, 1, 128], F32, tag="qb")
                nc.gpsimd.partition_broadcast(
                    qbc, q16_sb[0:1, p * 128 : (p + 1) * 128]
                )

                # scores = per-chunk sum_d K[s',(c,d)] * q[d]   (VectorE, fp32)
                kv3 = kv[:, 0:s].rearrange("p (c d) -> p c d", d=128)
                prod = pdpool.tile([128, ns, 128], F32, tag="pd")
                nc.vector.tensor_mul(prod, kv3, qbc.to_broadcast([128, ns, 128]))
                sc = prpool.tile([128, ns], F32, tag="sc")
                nc.vector.tensor_reduce(sc, prod, axis=AX.X, op=ALU.add)

                pr = prpool.tile([128, ns], F32, tag="pr")
                nc.scalar.activation(out=pr, in_=sc, func=AF.Exp)

                # y row (1,129): 128 dims + denominator via ones column
                yps = ypsum.tile([1, 129], F32, tag="y")
                for c in range(ns):
                    nc.tensor.matmul(
                        yps,
                        lhsT=pr[:, c : c + 1],
                        rhs=kv[:, s + c * 129 : s + (c + 1) * 129],
                        start=(c == 0),
                        stop=(c == ns - 1),
                    )
                nc.scalar.copy(out=yall[0:1, p, :], in_=yps)
                # transpose y row into column p of the accumulator
                nc.tensor.matmul(
                    attnT_ps[:, p : p + 1],
                    lhsT=yall[0:1, p, 0:128],
                    rhs=ones_one,
                    start=True,
                    stop=True,
                )
                if p == pairs // 2 - 1:
                    # overlap first half of the epilogue with the main loop
                    normalize_heads(0, h // 2)

            normalize_heads(h // 2, h)

            # residual + layernorm
            y_res = epi.tile([bl, 128], F32)
            nc.vector.tensor_add(y_res, oproj_ps, xpb_sb)

            stats = epi.tile([bl, 6], F32)
            nc.vector.bn_stats(out=stats, in_=y_res)
            mv = epi.tile([bl, 2], F32)
            nc.vector.bn_aggr(out=mv, in_=stats)
            std = epi.tile([bl, 1], F32)
            nc.scalar.activation(out=std, in_=mv[:, 1:2], func=AF.Sqrt, bias=eps_sb, scale=1.0)
            rstd = epi.tile([bl, 1], F32)
            nc.vector.reciprocal(rstd, std)
            outp = epi.tile([bl, 128], F32)
            nc.vector.tensor_scalar(
                outp, y_res, mv[:, 0:1], rstd, ALU.subtract, ALU.mult
            )
            nc.sync.dma_start(out=out_d, in_=outp)

    nc.compile()
    return nc


def host_prepare(x, past_k, past_v, Wqkv, bqkv, Wout, bout,
                 n_cores=N_CORES, bl=BL, h=H, ns=NS):
    """Pure numpy layout/projection work. Returns per-core input maps plus the
    host-computed k_new/v_new for output assembly."""
    s = ns * 128
    pairs = h * bl
    d = 128
    hd = h * d
    scale = np.float32(1.0 / math.sqrt(d))

    x2 = np.ascontiguousarray(x.reshape(n_cores * bl, d).astype(np.float32))

    # qkv projection on host (one token per batch; tiny)
    qkv = x2 @ Wqkv.astype(np.float32).T + bqkv.astype(np.float32)
    q = qkv[:, 0:hd].reshape(-1, h, d) * scale          # pre-scaled q
    k_new = qkv[:, hd : 2 * hd].reshape(-1, h, d)
    v_new = qkv[:, 2 * hd : 3 * hd].reshape(-1, h, d)
    enew = np.exp((q * k_new).sum(-1, dtype=np.float32)).astype(np.float32)  # (B, h)

    # WoutT[dd, hh, d'] = Wout[d', hh*128+dd]
    woutT = np.ascontiguousarray(
        Wout.astype(np.float32).reshape(d, h, d).transpose(2, 1, 0).reshape(d, h * d)
    )

    # blob: K chunked + V chunked with ones column, both (s' on partitions)
    blob = np.empty((n_cores, pairs, 128, s + ns * 129), np.float32)
    pk = past_k.astype(np.float32).reshape(n_cores, bl, h, ns, 128, d)
    pv = past_v.astype(np.float32).reshape(n_cores, bl, h, ns, 128, d)
    blob[..., :s].reshape(n_cores, pairs, 128, ns, d)[:] = (
        pk.transpose(0, 2, 1, 4, 3, 5).reshape(n_cores, pairs, 128, ns, d)
    )
    vx = blob[..., s:].reshape(n_cores, pairs, 128, ns, 129)
    vx[..., :d] = pv.transpose(0, 2, 1, 4, 3, 5).reshape(n_cores, pairs, 128, ns, d)
    vx[..., d] = 1.0

    in_maps = []
    for i in range(n_cores):
        sl = slice(i * bl, (i + 1) * bl)
        # pair index p = hh*bl + b
        q_c = np.ascontiguousarray(q[sl].transpose(1, 0, 2).reshape(pairs, d))
        vn_c = np.ascontiguousarray(v_new[sl].transpose(2, 1, 0).reshape(d, pairs))
        en_c = np.ascontiguousarray(enew[sl].T.reshape(1, pairs))
        in_maps.append({
            "kvblob": blob[i],
            "q16": q_c,
            "vnewT": vn_c,
            "enew": en_c,
            "woutT": woutT,
            "xpb": np.ascontiguousarray(x2[sl] + bout.astype(np.float32)),
        })
    return in_maps, k_new, v_new


def assemble_outputs(results, past_k, past_v, k_new, v_new,
                     n_cores=N_CORES, bl=BL):
    out = np.empty((n_cores * bl, 1, 128), np.float32)
    for i in range(n_cores):
        out[i * bl : (i + 1) * bl, 0, :] = results[i]["out"]
    k = np.concatenate([past_k.astype(np.float32), k_new[:, :, None, :]], axis=2)
    v = np.concatenate([past_v.astype(np.float32), v_new[:, :, None, :]], axis=2)
    return out, k, v


_NC_CACHE = {}


def kernel(x, past_k, past_v, Wqkv, bqkv, Wout, bout, _trace=False):
    x = np.asarray(x); past_k = np.asarray(past_k); past_v = np.asarray(past_v)
    Wqkv = np.asarray(Wqkv); bqkv = np.asarray(bqkv)
    Wout = np.asarray(Wout); bout = np.asarray(bout)

    if "nc" not in _NC_CACHE:
        _NC_CACHE["nc"] = build_program()
    nc = _NC_CACHE["nc"]

    in_maps, k_new, v_new = host_prepare(x, past_k, past_v, Wqkv, bqkv, Wout, bout)
    res = run_bass_kernel_spmd(
        nc, in_maps, core_ids=list(range(N_CORES)), trace=_trace
    )
    out, k, v = assemble_outputs(res.results, past_k, past_v, k_new, v_new)
    if _trace:
        kernel.last_exec_time_ns = res.exec_time_ns
        kernel.last_results = res
    return out, k, v


# revision 14
# speedup vs baseline: 1.1417x; 1.1417x over previous
"""Trainium2 Bass kernel for single-token causal self-attention with KV cache.

Problem (hardcoded per spec): B=32, S=2048 cached positions, D=128 head dim
(= d_model), H=16 heads.  Returns (out, k, v) like the reference.

Sharding: data parallel over batch, 4 batches/core on 8 cores; no cross-core
communication.

Device kernel structure (per core, 64 (head,batch) pairs):
  - One contiguous 2.06 MB DMA per pair brings in K chunked (s on partitions)
    and V chunked with a ones-column appended per chunk (for the softmax
    denominator).
  - Scores on the VectorEngine: elementwise K * broadcast(q) (fp32, exact)
    then a per-chunk free-dim reduce -> scores (128 x 16).  This keeps the
    K pass off the TensorEngine, whose fp32 weight-load path is the
    bottleneck (~150 GB/s).
  - exp on ScalarE; V pass on TensorE with probs as the (1-column) stationary
    operand and V streaming through the moving side (~300 GB/s fp32), giving
    unnormalized y rows plus the denominator via the ones column.
  - A 1-column outer-product matmul transposes each y row into a column
    accumulator; a vectorized epilogue folds in the new token exactly
    (e_new, v_new from host), normalizes, out-projects, adds the residual
    and applies layernorm.
The tiny projections (qkv of one token, 25 MFLOP) and the k/v cache
concatenation are host-side numpy; all S-scaling work runs on device.
"""

import math

import numpy as np

import concourse.bacc as bacc
import concourse.bass as bass
import concourse.tile as tile
from concourse import mybir
from concourse.bass_utils import run_bass_kernel_spmd

F32 = mybir.dt.float32
AF = mybir.ActivationFunctionType
ALU = mybir.AluOpType
AX = mybir.AxisListType

# Problem dims
B, S, D, H = 32, 2048, 128, 16
N_CORES = 8
BL = B // N_CORES          # local batches per core
PAIRS = H * BL             # (head, batch) pairs per core
NS = S // 128              # 128-wide chunks of cached sequence
EPS = 1e-5


def build_program(bl=BL, h=H, ns=NS):
    """Emit the per-core Tile program. Parameterized so a scaled-down variant
    can be checked in CoreSim."""
    pairs = h * bl
    s = ns * 128
    sv = s + ns * 129          # K region | V_ext region (129-wide chunks)

    nc = bacc.Bacc(
        "TRN2", target_bir_lowering=False, debug=False, num_devices=N_CORES
    )

    kvblob = nc.dram_tensor("kvblob", [pairs, 128, sv], F32, kind="ExternalInput").ap()
    q16 = nc.dram_tensor("q16", [pairs, 128], F32, kind="ExternalInput").ap()
    vnewT = nc.dram_tensor("vnewT", [128, pairs], F32, kind="ExternalInput").ap()
    enew = nc.dram_tensor("enew", [1, pairs], F32, kind="ExternalInput").ap()
    woutT = nc.dram_tensor("woutT", [128, h * 128], F32, kind="ExternalInput").ap()
    xpb = nc.dram_tensor("xpb", [bl, 128], F32, kind="ExternalInput").ap()
    out_d = nc.dram_tensor("out", [bl, 128], F32, kind="ExternalOutput").ap()

    with tile.TileContext(nc) as tc:
        with (
            tc.tile_pool(name="const", bufs=1) as const,
            tc.tile_pool(name="kv", bufs=7) as kvpool,
            tc.tile_pool(name="qb", bufs=3) as qbpool,
            tc.tile_pool(name="pd", bufs=2) as pdpool,
            tc.tile_pool(name="pr", bufs=3) as prpool,
            tc.tile_pool(name="epi", bufs=1) as epi,
            tc.tile_pool(name="ypsum", bufs=3, space="PSUM") as ypsum,
            tc.tile_pool(name="mpsum", bufs=2, space="PSUM") as mpsum,
            tc.tile_pool(name="apsum", bufs=1, space="PSUM") as apsum,
        ):
            # ---- constants / small inputs (SWDGE: keep the HWDGE ring for
            # the big blob loads) ----
            q16_sb = const.tile([1, pairs * 128], F32)
            nc.gpsimd.dma_start(out=q16_sb, in_=q16.rearrange("p d -> (p d)")[None, :])
            woutT_sb = const.tile([128, h * 128], F32)
            nc.gpsimd.dma_start(out=woutT_sb, in_=woutT)
            xpb_sb = const.tile([bl, 128], F32)
            nc.gpsimd.dma_start(out=xpb_sb, in_=xpb)
            vnewT_sb = const.tile([128, pairs], F32)
            nc.gpsimd.dma_start(out=vnewT_sb, in_=vnewT)
            enew_sb = const.tile([1, pairs], F32)
            nc.gpsimd.dma_start(out=enew_sb, in_=enew)
            ones_one = const.tile([1, 1], F32)
            nc.vector.memset(ones_one, 1.0)
            ones_row = const.tile([1, 128], F32)
            nc.vector.memset(ones_row, 1.0)
            eps_sb = const.tile([bl, 1], F32)
            nc.vector.memset(eps_sb, EPS)

            # unnormalized y rows + denominators, one 129-slot per pair
            yall = const.tile([1, pairs, 129], F32)
            # y^T column accumulators, one per half so the first half can
            # be normalized (DVE reads) while PE still writes the second
            attnT_a = apsum.tile([128, pairs // 2], F32, tag="attnT_a")
            attnT_b = apsum.tile([128, pairs // 2], F32, tag="attnT_b")

            attn_sb = const.tile([128, pairs], F32)
            oproj_ps = apsum.tile([bl, 128], F32, tag="oproj")

            def normalize_heads(h0, h1):
                """Fold in new token + normalize + out-project heads [h0,h1)."""
                lo, hi = h0 * bl, h1 * bl
                n = hi - lo
                att = attnT_a if h0 == 0 else attnT_b
                dtot = epi.tile([1, n], F32, tag="dtot")
                nc.vector.tensor_add(dtot, yall[0:1, lo:hi, 128], enew_sb[0:1, lo:hi])
                brsrc = epi.tile([1, 2 * n], F32, tag="brsrc")
                nc.vector.reciprocal(brsrc[0:1, 0:n], dtot)
                nc.vector.tensor_mul(
                    brsrc[0:1, n : 2 * n], enew_sb[0:1, lo:hi], brsrc[0:1, 0:n]
                )
                bc_ps = mpsum.tile([128, 2 * n], F32, tag="m")
                nc.tensor.matmul(bc_ps, lhsT=ones_row, rhs=brsrc, start=True, stop=True)
                bc_sb = epi.tile([128, 2 * n], F32, tag="bc")
                nc.vector.tensor_copy(bc_sb, bc_ps)
                t1 = epi.tile([128, n], F32, tag="t1")
                nc.vector.tensor_mul(t1, att[:, 0:n], bc_sb[:, 0:n])
                t2 = epi.tile([128, n], F32, tag="t2")
                nc.vector.tensor_mul(t2, vnewT_sb[:, lo:hi], bc_sb[:, n : 2 * n])
                nc.vector.tensor_add(attn_sb[:, lo:hi], t1, t2)
                for hh in range(h0, h1):
                    nc.tensor.matmul(
                        oproj_ps,
                        lhsT=attn_sb[:, hh * bl : (hh + 1) * bl],
                        rhs=woutT_sb[:, hh * 128 : (hh + 1) * 128],
                        start=(hh == 0),
                        stop=(hh == h - 1),
                    )

            # ---- main loop over (head, batch) pairs ----
            for p in range(pairs):
                kv = kvpool.tile([128, sv], F32, tag="kv")
                if p >= pairs - 2:
                    # latency-split the last loads so compute drains sooner
                    nc.sync.dma_start(out=kv[:, 0:s], in_=kvblob[p, :, 0:s])
                    nc.sync.dma_start(out=kv[:, s:sv], in_=kvblob[p, :, s:sv])
                else:
                    nc.sync.dma_start(out=kv, in_=kvblob[p])

                # q broadcast from partition 0 to all partitions (GpSimd)
                qbc = qbpool.tile([128, 1, 128], F32, tag="qb")
                nc.gpsimd.partition_broadcast(
                    qbc, q16_sb[0:1, p * 128 : (p + 1) * 128]
                )

                # scores = per-chunk sum_d K[s',(c,d)] * q[d]   (VectorE, fp32)
                kv3 = kv[:, 0:s].rearrange("p (c d) -> p c d", d=128)
                prod = pdpool.tile([128, ns, 128], F32, tag="pd")
                nc.vector.tensor_mul(prod, kv3, qbc.to_broadcast([128, ns, 128]))
                sc = prpool.tile([128, ns], F32, tag="sc")
                nc.vector.tensor_reduce(sc, prod, axis=AX.X, op=ALU.add)

                pr = prpool.tile([128, ns], F32, tag="pr")
                nc.scalar.activation(out=pr, in_=sc, func=AF.Exp)

                # y row (1,129): 128 dims + denominator via ones column
                yps = ypsum.tile([1, 129], F32, tag="y")
                for c in range(ns):
                    nc.tensor.matmul(
                        yps,
                        lhsT=pr[:, c : c + 1],
                        rhs=kv[:, s + c * 129 : s + (c + 1) * 129],
                        start=(c == 0),
                        stop=(c == ns - 1),
                    )
                nc.scalar.copy(out=yall[0:1, p, :], in_=yps)
                # transpose y row into column p of the accumulator
                att, pc = (attnT_a, p) if p < pairs // 2 else (attnT_b, p - pairs // 2)
                nc.tensor.matmul(
                    att[:, pc : pc + 1],
                    lhsT=yall[0:1, p, 0:128],
                    rhs=ones_one,
                    start=True,
                    stop=True,
                )
                if p == pairs // 2 - 1:
                    # overlap first half of the epilogue with the main loop
                    normalize_heads(0, h // 2)

            normalize_heads(h // 2, h)

            # residual + layernorm
            y_res = epi.tile([bl, 128], F32)
            nc.vector.tensor_add(y_res, oproj_ps, xpb_sb)

            stats = epi.tile([bl, 6], F32)
            nc.vector.bn_stats(out=stats, in_=y_res)
            mv = epi.tile([bl, 2], F32)
            nc.vector.bn_aggr(out=mv, in_=stats)
            std = epi.tile([bl, 1], F32)
            nc.scalar.activation(out=std, in_=mv[:, 1:2], func=AF.Sqrt, bias=eps_sb, scale=1.0)
            rstd = epi.tile([bl, 1], F32)
            nc.vector.reciprocal(rstd, std)
            outp = epi.tile([bl, 128], F32)
            nc.vector.tensor_scalar(
                outp, y_res, mv[:, 0:1], rstd, ALU.subtract, ALU.mult
            )
            nc.sync.dma_start(out=out_d, in_=outp)

    nc.compile()
    return nc


def host_prepare(x, past_k, past_v, Wqkv, bqkv, Wout, bout,
                 n_cores=N_CORES, bl=BL, h=H, ns=NS):
    """Pure numpy layout/projection work. Returns per-core input maps plus the
    host-computed k_new/v_new for output assembly."""
    s = ns * 128
    pairs = h * bl
    d = 128
    hd = h * d
    scale = np.float32(1.0 / math.sqrt(d))

    x2 = np.ascontiguousarray(x.reshape(n_cores * bl, d).astype(np.float32))

    # qkv projection on host (one token per batch; tiny)
    qkv = x2 @ Wqkv.astype(np.float32).T + bqkv.astype(np.float32)
    q = qkv[:, 0:hd].reshape(-1, h, d) * scale          # pre-scaled q
    k_new = qkv[:, hd : 2 * hd].reshape(-1, h, d)
    v_new = qkv[:, 2 * hd : 3 * hd].reshape(-1, h, d)
    enew = np.exp((q * k_new).sum(-1, dtype=np.float32)).astype(np.float32)  # (B, h)

    # WoutT[dd, hh, d'] = Wout[d', hh*128+dd]
    woutT = np.ascontiguousarray(
        Wout.astype(np.float32).reshape(d, h, d).transpose(2, 1, 0).reshape(d, h * d)
    )

    # blob: K chunked + V chunked with ones column, both (s' on partitions)
    blob = np.empty((n_cores, pairs, 128, s + ns * 129), np.float32)
    pk = past_k.astype(np.float32).reshape(n_cores, bl, h, ns, 128, d)
    pv = past_v.astype(np.float32).reshape(n_cores, bl, h, ns, 128, d)
    blob[..., :s].reshape(n_cores, pairs, 128, ns, d)[:] = (
        pk.transpose(0, 2, 1, 4, 3, 5).reshape(n_cores, pairs, 128, ns, d)
    )
    vx = blob[..., s:].reshape(n_cores, pairs, 128, ns, 129)
    vx[..., :d] = pv.transpose(0, 2, 1, 4, 3, 5).reshape(n_cores, pairs, 128, ns, d)
    vx[..., d] = 1.0

    in_maps = []
    for i in range(n_cores):
        sl = slice(i * bl, (i + 1) * bl)
        # pair index p = hh*bl + b
        q_c = np.ascontiguousarray(q[sl].transpose(1, 0, 2).reshape(pairs, d))
        vn_c = np.ascontiguousarray(v_new[sl].transpose(2, 1, 0).reshape(d, pairs))
        en_c = np.ascontiguousarray(enew[sl].T.reshape(1, pairs))
        in_maps.append({
            "kvblob": blob[i],
            "q16": q_c,
            "vnewT": vn_c,
            "enew": en_c,
            "woutT": woutT,
            "xpb": np.ascontiguousarray(x2[sl] + bout.astype(np.float32)),
        })
    return in_maps, k_new, v_new


def assemble_outputs(results, past_k, past_v, k_new, v_new,
                     n_cores=N_CORES, bl=BL):
    out = np.empty((n_cores * bl, 1, 128), np.float32)
    for i in range(n_cores):
        out[i * bl : (i + 1) * bl, 0, :] = results[i]["out"]
    k = np.concatenate([past_k.astype(np.float32), k_new[:, :, None, :]], axis=2)
    v = np.concatenate([past_v.astype(np.float32), v_new[:, :, None, :]], axis=2)
    return out, k, v


_NC_CACHE = {}


def kernel(x, past_k, past_v, Wqkv, bqkv, Wout, bout, _trace=False):
    x = np.asarray(x); past_k = np.asarray(past_k); past_v = np.asarray(past_v)
    Wqkv = np.asarray(Wqkv); bqkv = np.asarray(bqkv)
    Wout = np.asarray(Wout); bout = np.asarray(bout)

    if "nc" not in _NC_CACHE:
        _NC_CACHE["nc"] = build_program()
    nc = _NC_CACHE["nc"]

    in_maps, k_new, v_new = host_prepare(x, past_k, past_v, Wqkv, bqkv, Wout, bout)
    res = run_bass_kernel_spmd(
        nc, in_maps, core_ids=list(range(N_CORES)), trace=_trace
    )
    out, k, v = assemble_outputs(res.results, past_k, past_v, k_new, v_new)
    if _trace:
        kernel.last_exec_time_ns = res.exec_time_ns
        kernel.last_results = res
    return out, k, v
